# revision 33
# baseline (speedup 1.0000x reference)
"""Trainium2 Bass kernel for the ButterflyMlp problem.

Computes log_softmax(L3(relu(L2(relu(L1(x)))))) where each Li is a masked
linear layer (butterfly sparsity: global column stripes + a diagonal band),
batch 65536, data-parallel over 8 NeuronCores (8192 rows/core).

Strategy (per core, feature-major throughout):
  - Masks are pre-applied to weights on host. Layer-1 exploits the butterfly
    structure: the stripe columns (mask true for every output row) form a
    dense [|S|, 784] GEMM shared by all outputs, and the per-output-block
    band adds one narrow [|R_j|<=128, 128] GEMM per 128-row output block.
  - Layers 1 and 2 run in fp8-e4m3 (x, masked W1, y1, and W2 quantized;
    W1 scaled by 16 and W2 by 8 to stay in fp8 normal range, the combined
    x128 undone for free by the ACT scale on the y2 eviction — exact linear
    rescale, no extra ops). The 204 stripe columns contract in ONE matmul
    per output block via DoubleRow perf mode (2 fp8 k-rows per PE cell),
    halving the stripe matmul count, and layer-2's 128-wide k-chunks pair
    up the same way (4 matmuls instead of 7). Host-measured end-to-end
    error of the full quantization is ~1.1e-2 vs the 2e-2 budget. fp8 also
    halves the x DMA stream (8.1 MB/core), which otherwise outruns SWDGE
    and stalls the PE.
  - PSUM tiles are 2 banks wide ([*, 1024]); each relu+bias eviction covers
    a 1024-column chunk pair, halving ACT/DVE per-op overhead.
  - log_softmax is batched 4 chunks at a time: the four [10, 512] layer-3
    GEMMs are col-tiled (tile_position=(0, 32*t)) into one PSUM bank at
    partition offsets 0/32/64/96 and run concurrently; b3 is accumulated
    into the same bank by a K=1 ones-matmul; one block-diagonal ones-matmul
    computes all four groups' sum(exp) at once. Exp/Ln then process
    [106, 512] (4 chunks) per op instead of [10, 512] per chunk, and the
    final subtract reads PSUM directly (no y3 eviction op at all).
  - All x data for a superchunk moves in ONE SWDGE transfer; the first two
    superchunks are small (512 cols) so the first matmul fires early.
"""
import sys
sys.path.insert(0, "/opt/trn_rl_repo")
import numpy as np
import ml_dtypes

import concourse.bass as bass
import concourse.bacc as bacc
import concourse.mybir as mybir
import concourse.tile as tile
from concourse import bass_utils

F32 = mybir.dt.float32
F16 = mybir.dt.float16
F8 = mybir.dt.float8e4
NP8 = ml_dtypes.float8_e4m3
AF = mybir.ActivationFunctionType
ALU = mybir.AluOpType
DR = mybir.MatmulPerfMode.DoubleRow

# All activation functions this kernel uses live together in the
# natural_log_exp_and_others table set, but the greedy per-function set
# chooser picks exp_and_others for Exp and natural_log* for Ln, reloading
# ACT tables twice per chunk (~1.3us each). Restrict every other set's
# advertised contents so the chooser lands on the one set that covers
# everything and emits a single load. Set ids stay valid: the dict keys
# and order are unchanged.
_PIN_SET = "natural_log_exp_and_others"
_orig_gat = bacc.get_activation_tables


def _pinned_gat(arch):
    tabs = _orig_gat(arch)
    need = {AF.Relu, AF.Identity, AF.Exp, AF.Ln, AF.Copy}
    if _PIN_SET in tabs and need <= tabs[_PIN_SET]:
        for name in tabs:
            if name != _PIN_SET:
                tabs[name] = tabs[name] - need
    return tabs


bacc.get_activation_tables = _pinned_gat

N_CORES = 8
NB = 512          # batch columns per matmul (one PSUM bank of fp32)
OT = 128          # layer-1 output block width (6x128 + 1x16)
SCS = [512, 512, 1024, 2048, 2048, 2048]   # superchunk widths (sum = Bc)
GRP = 4           # NB-chunks per log_softmax group (4 col-tile slots)
WSCALE = 16.0     # fp8 W1 pre-scale
W2SCALE = 8.0     # fp8 W2 pre-scale (y2 eviction divides by 16*8)


def _decompose_mask1(mask1):
    """Split the butterfly mask into stripe columns S (true for every row)
    and per-output-block residual columns R_j (blocks of OT rows)."""
    D_out, D_in = mask1.shape
    S = np.where(mask1.all(axis=0))[0]
    n_blk = (D_out + OT - 1) // OT
    stripe_set = np.zeros(D_in, dtype=bool)
    stripe_set[S] = True
    R_list = []
    for j in range(n_blk):
        blk = mask1[j * OT:(j + 1) * OT]
        cols = np.where(blk.any(axis=0) & ~stripe_set)[0]
        assert len(cols) <= 128, f"band block {j} has {len(cols)} cols"
        R_list.append(cols)
    return S, R_list


def _build_program(meta):
    nS, R_lens = meta["nS"], meta["R_lens"]
    P_pad = meta["P_pad"]
    Bc = meta["Bc"]
    D1, H, C = meta["D1"], meta["H"], meta["C"]
    n_blk = len(R_lens)
    blk_w = [min(OT, D1 - j * OT) for j in range(n_blk)]
    n_sc = (nS + 127) // 128              # stripe K-chunks
    sc_w = -(-nS // n_sc)                 # stripe chunk width (padded)
    use_dr = (n_sc == 2)                  # DoubleRow wants exactly 2 chunks
    n_lane = n_sc + n_blk                 # x lanes per superchunk slab
    n_ch = Bc // NB                       # NB chunks per core
    assert sum(SCS) == Bc
    loc = []                              # chunk -> (superchunk, local col)
    for s, S_w in enumerate(SCS):
        for co in range(0, S_w, NB):
            loc.append((s, co))
    assert len(loc) == n_ch
    EP = 32 * (GRP - 1) + C               # epilogue partition span (106)
    n_pr = n_blk // 2                     # layer-2 DoubleRow k-chunk pairs
    lw = blk_w[-1] if n_blk % 2 else 0    # leftover block width
    assert all(blk_w[2 * q] == OT and blk_w[2 * q + 1] == OT
               for q in range(n_pr)), "layer-2 DR pairs need full blocks"
    # block-(n_blk-1) fused stripe+band lane: its whole contraction
    # (nS + R_last <= 256 rows) fits ONE DoubleRow matmul
    fuse = bool(lw) and use_dr and nS + R_lens[-1] <= 256
    fP = -(-(nS + R_lens[-1]) // 2) if fuse else 0   # fused Ki
    if fuse:
        P_pad = max(P_pad, fP)
        n_lane += 1                       # fused lane occupies 2 slots

    nc = bacc.Bacc("TRN2", target_bir_lowering=False, debug=False,
                   enable_asserts=False, num_devices=N_CORES)

    x_d = [nc.dram_tensor(f"x{s}", [P_pad, n_lane, S_w], F8,
                          kind="ExternalInput").ap()
           for s, S_w in enumerate(SCS)]
    # fp8 weight slab: [ ws | wb | w2 | w26 | wf (fused block) ]
    ws_off, wb_off = 0, n_sc * D1
    w2_off = wb_off + D1
    w26_off = w2_off + 2 * n_pr * H
    wf_off = w26_off + (H if lw else 0)
    wall_cols = wf_off + (2 * lw if fuse else 0)
    wall_d = nc.dram_tensor("wall8", [128, wall_cols], F8,
                            kind="ExternalInput").ap()
    w3_d = nc.dram_tensor("w3", [H, C], F16, kind="ExternalInput").ap()
    b1_d = nc.dram_tensor("b1", [OT, n_blk], F32, kind="ExternalInput").ap()
    b2_d = nc.dram_tensor("b2", [H, 1], F32, kind="ExternalInput").ap()
    b3r_d = nc.dram_tensor("b3r", [1, EP], F16, kind="ExternalInput").ap()
    onec_d = nc.dram_tensor("onec", [1, NB], F16, kind="ExternalInput").ap()
    obd_d = nc.dram_tensor("obd", [EP, EP], F16, kind="ExternalInput").ap()
    out_d = nc.dram_tensor("out", [C, Bc], F16, kind="ExternalOutput").ap()

    with tile.TileContext(nc) as tc:
        with tc.tile_pool(name="wp", bufs=1) as wp, \
             tc.tile_pool(name="xp", bufs=len(SCS)) as xp, \
             tc.tile_pool(name="hp", bufs=2) as hp, \
             tc.tile_pool(name="yp", bufs=4) as yp, \
             tc.tile_pool(name="ep", bufs=2) as ep, \
             tc.tile_pool(name="ps1", bufs=2, space="PSUM") as ps1, \
             tc.tile_pool(name="ps2", bufs=1, space="PSUM") as ps2, \
             tc.tile_pool(name="ps3", bufs=1, space="PSUM") as ps3, \
             tc.tile_pool(name="ps4", bufs=1, space="PSUM") as ps4:

            # ---- all fp8 weights first on SWDGE in ONE transfer (the
            # HWDGE rings emit per-partition ~300 B packets at ~2.6 GB/s
            # for these strided slabs — measured 10-25 us there)
            wall_sb = wp.tile([128, wall_cols], F8)
            nc.gpsimd.dma_start(wall_sb[:], wall_d[:])
            ws8_sb = wall_sb[:sc_w, ws_off:ws_off + n_sc * D1].rearrange(
                "p (c d) -> p c d", c=n_sc)
            wb8_sb = wall_sb[:P_pad, wb_off:wb_off + D1]
            w2_sb = wall_sb[:OT, w2_off:w2_off + 2 * n_pr * H].rearrange(
                "p (o d) -> p o d", o=2)
            w26_sb = wall_sb[:lw, w26_off:w26_off + H] if lw else None
            wf_sb = (wall_sb[:fP, wf_off:wf_off + 2 * lw].rearrange(
                "p (o d) -> p o d", o=2) if fuse else None)

            # ---- every superchunk's x load: ONE packed SWDGE DMA each,
            # all resident simultaneously (73.7 KB/partition total) so the
            # stream never stalls on a tile-slot release.
            x_tiles = []
            for s, S_w in enumerate(SCS):
                xt = xp.tile([P_pad, n_lane, S_w], F8, name="xt", tag="xt")
                nc.gpsimd.dma_start(xt[:], x_d[s][:])
                x_tiles.append(xt)

            w3_sb = wp.tile([H, C], F16)
            nc.sync.dma_start(w3_sb[:], w3_d[:])
            b1_sb = wp.tile([OT, n_blk], F32)
            nc.sync.dma_start(b1_sb[:], b1_d[:])
            b2_sb = wp.tile([H, 1], F32)
            nc.sync.dma_start(b2_sb[:], b2_d[:])
            b3r_sb = wp.tile([1, EP], F16)
            nc.sync.dma_start(b3r_sb[:], b3r_d[:])
            onec_sb = wp.tile([1, NB], F16)
            nc.sync.dma_start(onec_sb[:], onec_d[:])
            obd_sb = wp.tile([EP, EP], F16)
            nc.sync.dma_start(obd_sb[:], obd_d[:])

            y2_tiles = []
            for p in range(n_ch // 2):
                t0, t1 = 2 * p, 2 * p + 1
                halves = [loc[t0], loc[t1]]

                # ---- layer 1: 7 output blocks, PSUM tile spans the pair;
                # evictions write fp8 into DoubleRow-paired k-chunk tiles
                y1p = [hp.tile([OT, 2, 2 * NB], F8, name=f"y1p_{q}",
                               tag=f"y1p{q}") for q in range(n_pr)]
                y1l = (hp.tile([lw, 2 * NB], F8, name="y1l", tag="y1l")
                       if lw else None)

                def l1_mms(dst_ap, j, s, co):
                    wj = blk_w[j]
                    xt = x_tiles[s]
                    if fuse and j == n_blk - 1:
                        nc.tensor.matmul(
                            dst_ap, wf_sb[:, :, 0:lw],
                            xt[:fP, n_sc + n_blk - 1:n_sc + n_blk + 1,
                               co:co + NB],
                            start=True, stop=True, perf_mode=DR)
                        return
                    if use_dr:
                        nc.tensor.matmul(dst_ap,
                                         ws8_sb[:, :, j * OT:j * OT + wj],
                                         xt[:sc_w, 0:n_sc, co:co + NB],
                                         start=True, stop=False, perf_mode=DR)
                    else:
                        for c in range(n_sc):
                            kw = nS - c * sc_w if c == n_sc - 1 else sc_w
                            nc.tensor.matmul(dst_ap,
                                             ws8_sb[:kw, c, j * OT:j * OT + wj],
                                             xt[:kw, c:c + 1, co:co + NB],
                                             start=(c == 0), stop=False)
                    nc.tensor.matmul(dst_ap,
                                     wb8_sb[:R_lens[j], j * OT:j * OT + wj],
                                     xt[:R_lens[j], n_sc + j, co:co + NB],
                                     start=False, stop=True)

                def evict(src_ap, j, dst_ap):
                    wj = blk_w[j]
                    if j % 2 == 0:
                        nc.vector.tensor_scalar(dst_ap, src_ap,
                                                b1_sb[:wj, j:j + 1], 0.0,
                                                op0=ALU.add, op1=ALU.max)
                    else:
                        nc.scalar.activation(dst_ap, src_ap, AF.Relu,
                                             bias=b1_sb[:wj, j:j + 1])

                def y1_dst(j, cols):
                    return (y1p[j // 2][:blk_w[j], j % 2, cols]
                            if j < 2 * n_pr else y1l[:, cols])

                if p == 0:
                    # two single-chunk pipelines: the very first matmuls
                    # only need superchunk 0 while the x stream still runs
                    for h, (s, co) in enumerate(halves):
                        for j in range(n_blk):
                            pt = ps1.tile([blk_w[j], NB], F32, tag="l1",
                                          name="p1")
                            l1_mms(pt[:], j, s, co)
                            evict(pt[:], j,
                                  y1_dst(j, slice(h * NB, (h + 1) * NB)))
                else:
                    for j in range(n_blk):
                        pt = ps1.tile([blk_w[j], 2 * NB], F32, tag="l1",
                                      name="p1")
                        for h, (s, co) in enumerate(halves):
                            l1_mms(pt[:, h * NB:(h + 1) * NB], j, s, co)
                        evict(pt[:], j, y1_dst(j, slice(None)))

                # ---- layer 2 (fp8 DoubleRow over k-chunk pairs) ----
                p2 = ps2.tile([H, 2 * NB], F32, tag="l2", name="p2")
                for q in range(n_pr):
                    for h in range(2):
                        nc.tensor.matmul(
                            p2[:, h * NB:(h + 1) * NB],
                            w2_sb[:, :, q * H:(q + 1) * H],
                            y1p[q][:, :, h * NB:(h + 1) * NB],
                            start=(q == 0), stop=(q == n_pr - 1 and not lw),
                            perf_mode=DR)
                if lw:
                    for h in range(2):
                        nc.tensor.matmul(
                            p2[:, h * NB:(h + 1) * NB], w26_sb[:],
                            y1l[:, h * NB:(h + 1) * NB],
                            start=False, stop=True)
                # undo the fp8 weight pre-scales (x16 from W1, x8 from W2)
                y2 = yp.tile([H, 2 * NB], F16, tag="y2")
                nc.scalar.activation(y2[:], p2[:], AF.Relu,
                                     bias=b2_sb[:, 0:1],
                                     scale=1.0 / (WSCALE * W2SCALE))
                y2_tiles.append(y2)

                # ---- batched layer 3 + log_softmax for 4 chunks, col-tiled
                # into one PSUM bank at partition offsets 0/32/64/96.
                # Emitted one pair LATE: the exp->lse->ln->sub chain is 3
                # serial engine hops, and emitting it right after its y2s
                # would stall the strict-FIFO ACT/DVE queues (and the L1
                # evictions queued behind it) for the chain's latency.
                # Deferring one pair lets independent eviction work fill
                # the queues while the chain's inputs are already ready.
                todo = []
                if p >= 2 and p % 2 == 0:
                    todo.append(p // 2 - 1)
                if p == n_ch // 2 - 1:
                    todo.append(n_ch // 4 - 1)
                for g in todo:
                    pg = ps3.tile([EP, NB], F32, tag="l3", name="pg")
                    # b3 broadcast into all slots via a K=1 ones-matmul
                    # (start=True clears the bank; col MMs accumulate).
                    nc.tensor.matmul(pg[:], b3r_sb[0:1, :], onec_sb[0:1, :],
                                     start=True, stop=False)
                    srcs = [(y2_tiles[2 * g], 0), (y2_tiles[2 * g], 1),
                            (y2_tiles[2 * g + 1], 0), (y2_tiles[2 * g + 1], 1)]
                    for tl, (y2t, h) in enumerate(srcs):
                        nc.tensor.matmul(
                            pg[32 * tl:32 * tl + C, :], w3_sb[:],
                            y2t[:, h * NB:(h + 1) * NB],
                            start=False, stop=(tl == 3),
                            tile_position=(0, 32 * tl))
                    ex = ep.tile([EP, NB], F16, tag="ex")
                    nc.scalar.activation(ex[:], pg[:], AF.Exp)
                    ps_l = ps4.tile([EP, NB], F32, tag="lse", name="ps_l")
                    nc.tensor.matmul(ps_l[:], obd_sb[:], ex[:], start=True,
                                     stop=True)
                    ls = ep.tile([EP, NB], F32, tag="ls")
                    nc.scalar.activation(ls[:], ps_l[:], AF.Ln)
                    o = ep.tile([EP, NB], F16, tag="o")
                    nc.vector.tensor_tensor(o[:], pg[:], ls[:],
                                            op=ALU.subtract)
                    for tl in range(GRP):
                        t = GRP * g + tl
                        ring = nc.sync if tl % 2 == 0 else nc.scalar
                        ring.dma_start(
                            out_d[:, t * NB:(t + 1) * NB],
                            o[32 * tl:32 * tl + C, :])

    nc.compile()
    return nc


_CACHE = {}


def _prepare(x, W1, b1, W2, b2, W3, b3, mask1, mask2, mask3):
    B, D1 = x.shape
    H = W2.shape[0]
    C = W3.shape[0]
    assert B % N_CORES == 0
    Bc = B // N_CORES

    S, R_list = _decompose_mask1(np.asarray(mask1))
    R_lens = [len(r) for r in R_list]
    n_blk = len(R_list)
    blk_w = [min(OT, D1 - j * OT) for j in range(n_blk)]
    P_pad = max(R_lens + [1])
    nS = len(S)
    n_sc = (nS + 127) // 128
    sc_w = -(-nS // n_sc)
    n_lane = n_sc + n_blk
    EP = 32 * (GRP - 1) + C

    Wm1 = (np.asarray(W1) * np.asarray(mask1)).astype(np.float32)
    Wm2 = (np.asarray(W2) * np.asarray(mask2)).astype(np.float32)
    Wm3 = (np.asarray(W3) * np.asarray(mask3)).astype(np.float32)

    c16 = lambda a: np.asarray(a, dtype=np.float16)
    c8 = lambda a: np.asarray(a, dtype=NP8)

    # ---- fp8 weight slab [ ws | wb | w2 | w26 | wf ] ----
    n_pr = n_blk // 2
    lw = blk_w[-1] if n_blk % 2 else 0
    use_dr = (n_sc == 2)
    fuse = bool(lw) and use_dr and nS + R_lens[-1] <= 256
    fP = -(-(nS + R_lens[-1]) // 2) if fuse else 0
    if fuse:
        P_pad = max(P_pad, fP)
        n_lane += 1
    ws_off, wb_off = 0, n_sc * D1
    w2_off = wb_off + D1
    w26_off = w2_off + 2 * n_pr * H
    wf_off = w26_off + (H if lw else 0)
    wall_cols = wf_off + (2 * lw if fuse else 0)
    wall = np.zeros((128, wall_cols), np.float32)
    for c in range(n_sc):
        rows = S[c * sc_w:(c + 1) * sc_w]
        wall[:len(rows), ws_off + c * D1:ws_off + (c + 1) * D1] = \
            Wm1[:, rows].T * WSCALE
    for j, R in enumerate(R_list):
        wall[:len(R), wb_off + j * OT:wb_off + j * OT + blk_w[j]] = \
            Wm1[j * OT:j * OT + blk_w[j], R].T * WSCALE
    for q in range(n_pr):
        for o in range(2):
            j = 2 * q + o
            wall[:blk_w[j], w2_off + o * n_pr * H + q * H:
                 w2_off + o * n_pr * H + (q + 1) * H] = \
                Wm2[:, j * OT:j * OT + blk_w[j]].T * W2SCALE
    if lw:
        wall[:lw, w26_off:w26_off + H] = Wm2[:, (n_blk - 1) * OT:].T * W2SCALE
    if fuse:
        frows = np.concatenate([S, R_list[-1]])
        Wlast = Wm1[(n_blk - 1) * OT:, :]                 # [lw, D1]
        for o in range(2):
            rows = frows[o * fP:(o + 1) * fP]
            wall[:len(rows), wf_off + o * lw:wf_off + (o + 1) * lw] = \
                Wlast[:, rows].T * WSCALE
    w3 = np.ascontiguousarray(Wm3.T)                      # [H, C]
    b1p = np.zeros((OT, n_blk), np.float32)
    for j in range(n_blk):
        b1p[:blk_w[j], j] = WSCALE * \
            np.asarray(b1, np.float32)[j * OT:j * OT + blk_w[j]]
    b2p = np.asarray(b2, np.float32).reshape(H, 1)
    b3r = np.zeros((1, EP), np.float32)
    obd = np.zeros((EP, EP), np.float32)
    for tl in range(GRP):
        b3r[0, 32 * tl:32 * tl + C] = np.asarray(b3, np.float32)
        for m in range(32 * tl, min(32 * tl + 32, EP)):
            obd[32 * tl:32 * tl + C, m] = 1.0
    onec = np.ones((1, NB), np.float32)

    # ---- x slabs: one array per superchunk [NC, P_pad, n_lane, S_w] ----
    xT = np.asarray(x, np.float32).T                      # [D1, B] view
    xarrs = [np.zeros((N_CORES, P_pad, n_lane, S_w), NP8) for S_w in SCS]

    def fill_lane(lane, rows):
        data = c8(xT[rows]).reshape(len(rows), N_CORES, Bc)
        start = 0
        for s, S_w in enumerate(SCS):
            xarrs[s][:, :len(rows), lane, :] = \
                data[:, :, start:start + S_w].transpose(1, 0, 2)
            start += S_w

    for c in range(n_sc):
        fill_lane(c, S[c * sc_w:(c + 1) * sc_w])
    n_band = n_blk - 1 if fuse else n_blk
    for j in range(n_band):
        fill_lane(n_sc + j, R_list[j])
    if fuse:
        for o in range(2):
            fill_lane(n_sc + n_blk - 1 + o, frows[o * fP:(o + 1) * fP])

    meta = dict(nS=nS, R_lens=R_lens, P_pad=P_pad, Bc=Bc, D1=D1, H=H, C=C)
    key = (B, D1, H, C, nS, tuple(R_lens))
    if key not in _CACHE:
        _CACHE[key] = _build_program(meta)
    nc = _CACHE[key]

    in_maps = []
    for cidx in range(N_CORES):
        m = {f"x{s}": xarrs[s][cidx] for s in range(len(SCS))}
        m.update({
            "wall8": c8(wall), "w3": c16(w3),
            "b1": b1p, "b2": b2p,
            "b3r": c16(b3r), "onec": c16(onec), "obd": c16(obd),
        })
        in_maps.append(m)
    return nc, in_maps, meta


def _assemble(results, meta):
    outs = [np.ascontiguousarray(results[c]["out"].T).astype(np.float32)
            for c in range(N_CORES)]
    return np.concatenate(outs, axis=0)


def kernel(**inputs):
    nc, in_maps, meta = _prepare(**inputs)
    res = bass_utils.run_bass_kernel_spmd(nc, in_maps,
                                          core_ids=list(range(N_CORES)))
    return _assemble(res.results, meta)


def kernel_traced(tmpdir=None, **inputs):
    """Same as kernel() but with NTFF profiling; returns (output, results)."""
    nc, in_maps, meta = _prepare(**inputs)
    res = bass_utils.run_bass_kernel_spmd(nc, in_maps,
                                          core_ids=list(range(N_CORES)),
                                          trace=True, tmpdir=tmpdir)
    return _assemble(res.results, meta), res


# revision 35
# speedup vs baseline: 1.4736x; 1.4736x over previous
"""Trainium2 Bass kernel for the ButterflyMlp problem.

Computes log_softmax(L3(relu(L2(relu(L1(x)))))) where each Li is a masked
linear layer (butterfly sparsity: global column stripes + a diagonal band),
batch 65536, data-parallel over 8 NeuronCores (8192 rows/core).

Strategy (per core, feature-major throughout):
  - Masks are pre-applied to weights on host. Layer-1 exploits the butterfly
    structure: the stripe columns (mask true for every output row) form a
    dense [|S|, 784] GEMM shared by all outputs, and the per-output-block
    band adds one narrow [|R_j|<=128, 128] GEMM per 128-row output block.
  - Layers 1 and 2 run in fp8-e4m3 (x, masked W1, y1, and W2 quantized;
    W1 scaled by 16 and W2 by 8 to stay in fp8 normal range, the combined
    x128 undone for free by the ACT scale on the y2 eviction — exact linear
    rescale, no extra ops). The 204 stripe columns contract in ONE matmul
    per output block via DoubleRow perf mode (2 fp8 k-rows per PE cell),
    halving the stripe matmul count, and layer-2's 128-wide k-chunks pair
    up the same way (4 matmuls instead of 7). Host-measured end-to-end
    error of the full quantization is ~1.1e-2 vs the 2e-2 budget. fp8 also
    halves the x DMA stream (8.1 MB/core), which otherwise outruns SWDGE
    and stalls the PE.
  - PSUM tiles are 2 banks wide ([*, 1024]); each relu+bias eviction covers
    a 1024-column chunk pair, halving ACT/DVE per-op overhead.
  - log_softmax is batched 4 chunks at a time: the four [10, 512] layer-3
    GEMMs are col-tiled (tile_position=(0, 32*t)) into one PSUM bank at
    partition offsets 0/32/64/96 and run concurrently; b3 is accumulated
    into the same bank by a K=1 ones-matmul; one block-diagonal ones-matmul
    computes all four groups' sum(exp) at once. Exp/Ln then process
    [106, 512] (4 chunks) per op instead of [10, 512] per chunk, and the
    final subtract reads PSUM directly (no y3 eviction op at all).
  - All x data for a superchunk moves in ONE SWDGE transfer; the first two
    superchunks are small (512 cols) so the first matmul fires early.
"""
import sys
sys.path.insert(0, "/opt/trn_rl_repo")
import numpy as np
import ml_dtypes

import concourse.bass as bass
import concourse.bacc as bacc
import concourse.mybir as mybir
import concourse.tile as tile
from concourse import bass_utils

F32 = mybir.dt.float32
F16 = mybir.dt.float16
F8 = mybir.dt.float8e4
NP8 = ml_dtypes.float8_e4m3
AF = mybir.ActivationFunctionType
ALU = mybir.AluOpType
DR = mybir.MatmulPerfMode.DoubleRow

# All activation functions this kernel uses live together in the
# natural_log_exp_and_others table set, but the greedy per-function set
# chooser picks exp_and_others for Exp and natural_log* for Ln, reloading
# ACT tables twice per chunk (~1.3us each). Restrict every other set's
# advertised contents so the chooser lands on the one set that covers
# everything and emits a single load. Set ids stay valid: the dict keys
# and order are unchanged.
_PIN_SET = "natural_log_exp_and_others"
_orig_gat = bacc.get_activation_tables


def _pinned_gat(arch):
    tabs = _orig_gat(arch)
    need = {AF.Relu, AF.Identity, AF.Exp, AF.Ln, AF.Copy}
    if _PIN_SET in tabs and need <= tabs[_PIN_SET]:
        for name in tabs:
            if name != _PIN_SET:
                tabs[name] = tabs[name] - need
    return tabs


bacc.get_activation_tables = _pinned_gat

N_CORES = 8
NB = 512          # batch columns per matmul (one PSUM bank of fp32)
OT = 128          # layer-1 output block width (6x128 + 1x16)
SCS = [512, 512] + [1024] * 7   # superchunk widths (sum = Bc); kept at
                                # <=1024 so a degraded SWDGE rate delays
                                # each consumer by at most one slab
GRP = 4           # NB-chunks per log_softmax group (4 col-tile slots)
WSCALE = 16.0     # fp8 W1 pre-scale
W2SCALE = 8.0     # fp8 W2 pre-scale (y2 eviction divides by 16*8)


def _decompose_mask1(mask1):
    """Split the butterfly mask into stripe columns S (true for every row)
    and per-output-block residual columns R_j (blocks of OT rows)."""
    D_out, D_in = mask1.shape
    S = np.where(mask1.all(axis=0))[0]
    n_blk = (D_out + OT - 1) // OT
    stripe_set = np.zeros(D_in, dtype=bool)
    stripe_set[S] = True
    R_list = []
    for j in range(n_blk):
        blk = mask1[j * OT:(j + 1) * OT]
        cols = np.where(blk.any(axis=0) & ~stripe_set)[0]
        assert len(cols) <= 128, f"band block {j} has {len(cols)} cols"
        R_list.append(cols)
    return S, R_list


def _build_program(meta):
    nS, R_lens = meta["nS"], meta["R_lens"]
    P_pad = meta["P_pad"]
    Bc = meta["Bc"]
    D1, H, C = meta["D1"], meta["H"], meta["C"]
    n_blk = len(R_lens)
    blk_w = [min(OT, D1 - j * OT) for j in range(n_blk)]
    n_sc = (nS + 127) // 128              # stripe K-chunks
    sc_w = -(-nS // n_sc)                 # stripe chunk width (padded)
    use_dr = (n_sc == 2)                  # DoubleRow wants exactly 2 chunks
    n_lane = n_sc + n_blk                 # x lanes per superchunk slab
    n_ch = Bc // NB                       # NB chunks per core
    assert sum(SCS) == Bc
    loc = []                              # chunk -> (superchunk, local col)
    for s, S_w in enumerate(SCS):
        for co in range(0, S_w, NB):
            loc.append((s, co))
    assert len(loc) == n_ch
    EP = 32 * (GRP - 1) + C               # epilogue partition span (106)
    n_pr = n_blk // 2                     # layer-2 DoubleRow k-chunk pairs
    lw = blk_w[-1] if n_blk % 2 else 0    # leftover block width
    assert all(blk_w[2 * q] == OT and blk_w[2 * q + 1] == OT
               for q in range(n_pr)), "layer-2 DR pairs need full blocks"

    nc = bacc.Bacc("TRN2", target_bir_lowering=False, debug=False,
                   enable_asserts=False, num_devices=N_CORES)

    x_d = [nc.dram_tensor(f"x{s}", [P_pad, n_lane, S_w], F8,
                          kind="ExternalInput").ap()
           for s, S_w in enumerate(SCS)]
    # fp8 weight slab: [ ws (n_sc*D1) | wb (D1) | w2 (2*n_pr*H) | w26 (H) ]
    ws_off, wb_off = 0, n_sc * D1
    w2_off = wb_off + D1
    w26_off = w2_off + 2 * n_pr * H
    wall_cols = w26_off + (H if lw else 0)
    wall_d = nc.dram_tensor("wall8", [128, wall_cols], F8,
                            kind="ExternalInput").ap()
    w3_d = nc.dram_tensor("w3", [H, C], F16, kind="ExternalInput").ap()
    b1_d = nc.dram_tensor("b1", [OT, n_blk], F32, kind="ExternalInput").ap()
    b2_d = nc.dram_tensor("b2", [H, 1], F32, kind="ExternalInput").ap()
    b3r_d = nc.dram_tensor("b3r", [1, EP], F16, kind="ExternalInput").ap()
    onec_d = nc.dram_tensor("onec", [1, NB], F16, kind="ExternalInput").ap()
    obd_d = nc.dram_tensor("obd", [EP, EP], F16, kind="ExternalInput").ap()
    out_d = nc.dram_tensor("out", [C, Bc], F16, kind="ExternalOutput").ap()

    with tile.TileContext(nc) as tc:
        with tc.tile_pool(name="wp", bufs=1) as wp, \
             tc.tile_pool(name="xp", bufs=len(SCS)) as xp, \
             tc.tile_pool(name="hp", bufs=2) as hp, \
             tc.tile_pool(name="yp", bufs=4) as yp, \
             tc.tile_pool(name="ep", bufs=2) as ep, \
             tc.tile_pool(name="ps1", bufs=2, space="PSUM") as ps1, \
             tc.tile_pool(name="ps2", bufs=1, space="PSUM") as ps2, \
             tc.tile_pool(name="ps3", bufs=1, space="PSUM") as ps3, \
             tc.tile_pool(name="ps4", bufs=1, space="PSUM") as ps4:

            # ---- all fp8 weights first on SWDGE in ONE transfer (the
            # HWDGE rings emit per-partition ~300 B packets at ~2.6 GB/s
            # for these strided slabs — measured 10-25 us there)
            wall_sb = wp.tile([128, wall_cols], F8)
            nc.gpsimd.dma_start(wall_sb[:], wall_d[:])
            ws8_sb = wall_sb[:sc_w, ws_off:ws_off + n_sc * D1].rearrange(
                "p (c d) -> p c d", c=n_sc)
            wb8_sb = wall_sb[:P_pad, wb_off:wb_off + D1]
            w2_sb = wall_sb[:OT, w2_off:w2_off + 2 * n_pr * H].rearrange(
                "p (o d) -> p o d", o=2)
            w26_sb = wall_sb[:lw, w26_off:w26_off + H] if lw else None

            # ---- every superchunk's x load: ONE packed SWDGE DMA each,
            # all resident simultaneously (73.7 KB/partition total) so the
            # stream never stalls on a tile-slot release.
            x_tiles = []
            for s, S_w in enumerate(SCS):
                xt = xp.tile([P_pad, n_lane, S_w], F8, name="xt", tag="xt")
                nc.gpsimd.dma_start(xt[:], x_d[s][:])
                x_tiles.append(xt)

            w3_sb = wp.tile([H, C], F16)
            nc.sync.dma_start(w3_sb[:], w3_d[:])
            b1_sb = wp.tile([OT, n_blk], F32)
            nc.sync.dma_start(b1_sb[:], b1_d[:])
            b2_sb = wp.tile([H, 1], F32)
            nc.sync.dma_start(b2_sb[:], b2_d[:])
            b3r_sb = wp.tile([1, EP], F16)
            nc.sync.dma_start(b3r_sb[:], b3r_d[:])
            onec_sb = wp.tile([1, NB], F16)
            nc.sync.dma_start(onec_sb[:], onec_d[:])
            obd_sb = wp.tile([EP, EP], F16)
            nc.sync.dma_start(obd_sb[:], obd_d[:])

            y2_tiles = []
            for p in range(n_ch // 2):
                t0, t1 = 2 * p, 2 * p + 1
                halves = [loc[t0], loc[t1]]

                # ---- layer 1: 7 output blocks, PSUM tile spans the pair;
                # evictions write fp8 into DoubleRow-paired k-chunk tiles
                y1p = [hp.tile([OT, 2, 2 * NB], F8, name=f"y1p_{q}",
                               tag=f"y1p{q}") for q in range(n_pr)]
                y1l = (hp.tile([lw, 2 * NB], F8, name="y1l", tag="y1l")
                       if lw else None)

                def l1_mms(dst_ap, j, s, co):
                    wj = blk_w[j]
                    xt = x_tiles[s]
                    if use_dr:
                        nc.tensor.matmul(dst_ap,
                                         ws8_sb[:, :, j * OT:j * OT + wj],
                                         xt[:sc_w, 0:n_sc, co:co + NB],
                                         start=True, stop=False, perf_mode=DR)
                    else:
                        for c in range(n_sc):
                            kw = nS - c * sc_w if c == n_sc - 1 else sc_w
                            nc.tensor.matmul(dst_ap,
                                             ws8_sb[:kw, c, j * OT:j * OT + wj],
                                             xt[:kw, c:c + 1, co:co + NB],
                                             start=(c == 0), stop=False)
                    nc.tensor.matmul(dst_ap,
                                     wb8_sb[:R_lens[j], j * OT:j * OT + wj],
                                     xt[:R_lens[j], n_sc + j, co:co + NB],
                                     start=False, stop=True)

                def evict(src_ap, j, dst_ap):
                    wj = blk_w[j]
                    if j % 2 == 0:
                        nc.vector.tensor_scalar(dst_ap, src_ap,
                                                b1_sb[:wj, j:j + 1], 0.0,
                                                op0=ALU.add, op1=ALU.max)
                    else:
                        nc.scalar.activation(dst_ap, src_ap, AF.Relu,
                                             bias=b1_sb[:wj, j:j + 1])

                def y1_dst(j, cols):
                    return (y1p[j // 2][:blk_w[j], j % 2, cols]
                            if j < 2 * n_pr else y1l[:, cols])

                if p == 0:
                    # two single-chunk pipelines: the very first matmuls
                    # only need superchunk 0 while the x stream still runs
                    for h, (s, co) in enumerate(halves):
                        for j in range(n_blk):
                            pt = ps1.tile([blk_w[j], NB], F32, tag="l1",
                                          name="p1")
                            l1_mms(pt[:], j, s, co)
                            evict(pt[:], j,
                                  y1_dst(j, slice(h * NB, (h + 1) * NB)))
                else:
                    for j in range(n_blk):
                        pt = ps1.tile([blk_w[j], 2 * NB], F32, tag="l1",
                                      name="p1")
                        for h, (s, co) in enumerate(halves):
                            l1_mms(pt[:, h * NB:(h + 1) * NB], j, s, co)
                        evict(pt[:], j, y1_dst(j, slice(None)))

                # ---- layer 2 (fp8 DoubleRow over k-chunk pairs) ----
                p2 = ps2.tile([H, 2 * NB], F32, tag="l2", name="p2")
                for q in range(n_pr):
                    for h in range(2):
                        nc.tensor.matmul(
                            p2[:, h * NB:(h + 1) * NB],
                            w2_sb[:, :, q * H:(q + 1) * H],
                            y1p[q][:, :, h * NB:(h + 1) * NB],
                            start=(q == 0), stop=(q == n_pr - 1 and not lw),
                            perf_mode=DR)
                if lw:
                    for h in range(2):
                        nc.tensor.matmul(
                            p2[:, h * NB:(h + 1) * NB], w26_sb[:],
                            y1l[:, h * NB:(h + 1) * NB],
                            start=False, stop=True)
                # undo the fp8 weight pre-scales (x16 from W1, x8 from W2)
                y2 = yp.tile([H, 2 * NB], F16, tag="y2")
                nc.scalar.activation(y2[:], p2[:], AF.Relu,
                                     bias=b2_sb[:, 0:1],
                                     scale=1.0 / (WSCALE * W2SCALE))
                y2_tiles.append(y2)

                # ---- batched layer 3 + log_softmax for 4 chunks, col-tiled
                # into one PSUM bank at partition offsets 0/32/64/96.
                # Emitted one pair LATE: the exp->lse->ln->sub chain is 3
                # serial engine hops, and emitting it right after its y2s
                # would stall the strict-FIFO ACT/DVE queues (and the L1
                # evictions queued behind it) for the chain's latency.
                # Deferring one pair lets independent eviction work fill
                # the queues while the chain's inputs are already ready.
                todo = []
                if p >= 2 and p % 2 == 0:
                    todo.append(p // 2 - 1)
                if p == n_ch // 2 - 1:
                    todo.append(n_ch // 4 - 1)
                for g in todo:
                    pg = ps3.tile([EP, NB], F32, tag="l3", name="pg")
                    # b3 broadcast into all slots via a K=1 ones-matmul
                    # (start=True clears the bank; col MMs accumulate).
                    nc.tensor.matmul(pg[:], b3r_sb[0:1, :], onec_sb[0:1, :],
                                     start=True, stop=False)
                    srcs = [(y2_tiles[2 * g], 0), (y2_tiles[2 * g], 1),
                            (y2_tiles[2 * g + 1], 0), (y2_tiles[2 * g + 1], 1)]
                    for tl, (y2t, h) in enumerate(srcs):
                        nc.tensor.matmul(
                            pg[32 * tl:32 * tl + C, :], w3_sb[:],
                            y2t[:, h * NB:(h + 1) * NB],
                            start=False, stop=(tl == 3),
                            tile_position=(0, 32 * tl))
                    ex = ep.tile([EP, NB], F16, tag="ex")
                    nc.scalar.activation(ex[:], pg[:], AF.Exp)
                    ps_l = ps4.tile([EP, NB], F32, tag="lse", name="ps_l")
                    nc.tensor.matmul(ps_l[:], obd_sb[:], ex[:], start=True,
                                     stop=True)
                    ls = ep.tile([EP, NB], F32, tag="ls")
                    nc.scalar.activation(ls[:], ps_l[:], AF.Ln)
                    o = ep.tile([EP, NB], F16, tag="o")
                    nc.vector.tensor_tensor(o[:], pg[:], ls[:],
                                            op=ALU.subtract)
                    for tl in range(GRP):
                        t = GRP * g + tl
                        ring = nc.sync if tl % 2 == 0 else nc.scalar
                        ring.dma_start(
                            out_d[:, t * NB:(t + 1) * NB],
                            o[32 * tl:32 * tl + C, :])

    nc.compile()
    return nc


_CACHE = {}


def _prepare(x, W1, b1, W2, b2, W3, b3, mask1, mask2, mask3):
    B, D1 = x.shape
    H = W2.shape[0]
    C = W3.shape[0]
    assert B % N_CORES == 0
    Bc = B // N_CORES

    S, R_list = _decompose_mask1(np.asarray(mask1))
    R_lens = [len(r) for r in R_list]
    n_blk = len(R_list)
    blk_w = [min(OT, D1 - j * OT) for j in range(n_blk)]
    P_pad = max(R_lens + [1])
    nS = len(S)
    n_sc = (nS + 127) // 128
    sc_w = -(-nS // n_sc)
    n_lane = n_sc + n_blk
    EP = 32 * (GRP - 1) + C

    Wm1 = (np.asarray(W1) * np.asarray(mask1)).astype(np.float32)
    Wm2 = (np.asarray(W2) * np.asarray(mask2)).astype(np.float32)
    Wm3 = (np.asarray(W3) * np.asarray(mask3)).astype(np.float32)

    c16 = lambda a: np.asarray(a, dtype=np.float16)
    c8 = lambda a: np.asarray(a, dtype=NP8)

    # ---- fp8 weight slab [ ws | wb | w2 | w26 ] ----
    n_pr = n_blk // 2
    lw = blk_w[-1] if n_blk % 2 else 0
    ws_off, wb_off = 0, n_sc * D1
    w2_off = wb_off + D1
    w26_off = w2_off + 2 * n_pr * H
    wall_cols = w26_off + (H if lw else 0)
    wall = np.zeros((128, wall_cols), np.float32)
    for c in range(n_sc):
        rows = S[c * sc_w:(c + 1) * sc_w]
        wall[:len(rows), ws_off + c * D1:ws_off + (c + 1) * D1] = \
            Wm1[:, rows].T * WSCALE
    for j, R in enumerate(R_list):
        wall[:len(R), wb_off + j * OT:wb_off + j * OT + blk_w[j]] = \
            Wm1[j * OT:j * OT + blk_w[j], R].T * WSCALE
    for q in range(n_pr):
        for o in range(2):
            j = 2 * q + o
            wall[:blk_w[j], w2_off + o * n_pr * H + q * H:
                 w2_off + o * n_pr * H + (q + 1) * H] = \
                Wm2[:, j * OT:j * OT + blk_w[j]].T * W2SCALE
    if lw:
        wall[:lw, w26_off:w26_off + H] = Wm2[:, (n_blk - 1) * OT:].T * W2SCALE
    w3 = np.ascontiguousarray(Wm3.T)                      # [H, C]
    b1p = np.zeros((OT, n_blk), np.float32)
    for j in range(n_blk):
        b1p[:blk_w[j], j] = WSCALE * \
            np.asarray(b1, np.float32)[j * OT:j * OT + blk_w[j]]
    b2p = np.asarray(b2, np.float32).reshape(H, 1)
    b3r = np.zeros((1, EP), np.float32)
    obd = np.zeros((EP, EP), np.float32)
    for tl in range(GRP):
        b3r[0, 32 * tl:32 * tl + C] = np.asarray(b3, np.float32)
        for m in range(32 * tl, min(32 * tl + 32, EP)):
            obd[32 * tl:32 * tl + C, m] = 1.0
    onec = np.ones((1, NB), np.float32)

    # ---- x slabs: one array per superchunk [NC, P_pad, n_lane, S_w] ----
    xT = np.asarray(x, np.float32).T                      # [D1, B] view
    xarrs = [np.zeros((N_CORES, P_pad, n_lane, S_w), NP8) for S_w in SCS]

    def fill_lane(lane, rows):
        data = c8(xT[rows]).reshape(len(rows), N_CORES, Bc)
        start = 0
        for s, S_w in enumerate(SCS):
            xarrs[s][:, :len(rows), lane, :] = \
                data[:, :, start:start + S_w].transpose(1, 0, 2)
            start += S_w

    for c in range(n_sc):
        fill_lane(c, S[c * sc_w:(c + 1) * sc_w])
    for j, R in enumerate(R_list):
        fill_lane(n_sc + j, R)

    meta = dict(nS=nS, R_lens=R_lens, P_pad=P_pad, Bc=Bc, D1=D1, H=H, C=C)
    key = (B, D1, H, C, nS, tuple(R_lens))
    if key not in _CACHE:
        _CACHE[key] = _build_program(meta)
    nc = _CACHE[key]

    in_maps = []
    for cidx in range(N_CORES):
        m = {f"x{s}": xarrs[s][cidx] for s in range(len(SCS))}
        m.update({
            "wall8": c8(wall), "w3": c16(w3),
            "b1": b1p, "b2": b2p,
            "b3r": c16(b3r), "onec": c16(onec), "obd": c16(obd),
        })
        in_maps.append(m)
    return nc, in_maps, meta


def _assemble(results, meta):
    outs = [np.ascontiguousarray(results[c]["out"].T).astype(np.float32)
            for c in range(N_CORES)]
    return np.concatenate(outs, axis=0)


def kernel(**inputs):
    nc, in_maps, meta = _prepare(**inputs)
    res = bass_utils.run_bass_kernel_spmd(nc, in_maps,
                                          core_ids=list(range(N_CORES)))
    return _assemble(res.results, meta)


def kernel_traced(tmpdir=None, **inputs):
    """Same as kernel() but with NTFF profiling; returns (output, results)."""
    nc, in_maps, meta = _prepare(**inputs)
    res = bass_utils.run_bass_kernel_spmd(nc, in_maps,
                                          core_ids=list(range(N_CORES)),
                                          trace=True, tmpdir=tmpdir)
    return _assemble(res.results, meta), res
